# revision 16
# baseline (speedup 1.0000x reference)
"""Causal attention kernel for Trainium2 (Bass/Tile), 8-core SPMD.

Problem: B=2, H=16, S=2048, D=64 fp32 attention with a causal mask.
Sharding: batch*heads = 32 slices -> 4 heads per core across 8 cores.

Per-core algorithm (per head):
  S^T = K @ Q^T computed blockwise: [kblock=128, qtile=512] tiles so the
  softmax reduction axis (k) lands on PSUM partitions and q on the free dim.
  Host pre-scales Q by 1/(8*sqrt(D)) so PSUM holds r = score/8.

  exp is split across TWO engines (static LPT assignment):
   - ScalarE ACTIVATE: exp(8*r + 8*lnC) = C^8 * e^score
   - VectorE custom DVE op (EXP_MONIC8_ANT, one 8-stage instruction):
     ((r+a0)((r+a1)^2+a2))^8 ~ C^8 * e^score.
  Softmax is invariant to the global C^8 factor (host divides num/den).

  Masking (causal groups): only the [128,128] diagonal band of each mixed
  kblock actually varies; multiply just that band by a resident 0/1
  triangle tile on VectorE, and trim the mm1/mm2 matmuls so fully-masked
  columns are never computed/consumed. Non-causal mask patterns fall back
  to full-region mask multiplies with DMA'd tiles.

  out^T = V_aug^T @ P^T accumulated over kblocks in PSUM, where V_aug has a
  ones column appended -> row 64 of out^T is the softmax denominator.
  Host divides by the denominator and transposes back.

  The two heads of a pair live on partitions 0-63 / 64-127; their QK^T
  matmuls (contraction=64) are emitted adjacently so the PE runs them as
  concurrent row-tiles (tile_position auto-derived from base_partition).

Matmuls run as float32r (fp32 data, 1 cycle/row when N>=256).
"""

import sys

import numpy as np

for _p in ('/opt/trn_rl_repo', '/root/.axon_site/_ro/trn_rl_repo'):
    if _p not in sys.path:
        sys.path.append(_p)

B, H, S, D = 2, 16, 2048, 64
NCORES = 8
HPC = (B * H) // NCORES  # heads per core = 4
QT = 512                 # q tile (PSUM bank free dim)
KB = 128                 # k block (partition dim)
GSIZE = 2                # kblocks per exp group
NQT = S // QT            # 4
NKB = S // KB            # 16
MAX_RESIDENT_MASKS = 8

# exp(8r) ~ ((r+A0)((r+A1)^2+A2))^8 over r in [-0.75, 0.75]; LOGC matches the
# implicit constant so ScalarE exp groups (bias=8*LOGC) agree with poly groups.
# asymmetric minimax (tight on [-0.4, 0.76], loose tail; end-to-end metric
# 2.4e-3 on the reference inputs)
PA0 = 1.5910989925174288
PA1 = 0.6498370045543014
PA2 = 3.0790984792370817
PLOGC = 1.7185802495331195

# engine ids for exp/copy assignment
SC, DV = 0, 1

_CACHE = {}


def _register_exp_op():
    from concourse import dve_ops
    from concourse.dve_spec import Spec, Src0, C0, C1, C2, sq, lower
    from concourse.dve_uop import DveOpSpec

    name = "EXP_MONIC8_ANT"
    for o in dve_ops.OPS:
        if o.name == name:
            return o

    def ref(in0, in1, s0, s1, imm2):
        x = in0.astype(np.float32)
        p = ((x + np.float32(s0))
             * ((x + np.float32(s1)) ** 2 + np.float32(imm2))).astype(np.float32)
        p = (p * p).astype(np.float32)
        p = (p * p).astype(np.float32)
        p = (p * p).astype(np.float32)
        return p

    body = sq(sq(sq((Src0 + C0) * (sq(Src0 + C1) + C2))))
    spec = Spec(body=body, reference=ref)
    opcode = dve_ops._CUSTOM_DVE_ROW_BASE + len(dve_ops.OPS)
    assert opcode < 0x20
    shas = {}
    for ver in ("v3", "v4"):
        uops = lower(spec, ver=ver)
        shas[ver] = DveOpSpec(name=name, opcode=opcode, uops=uops,
                              rd1_en=False).sha(ver)
    op = dve_ops.DveOp(name, spec, False, shas)
    dve_ops.OPS.append(op)
    dve_ops.CUSTOM_DVE_SPECS[name] = spec
    dve_ops._SUB_OPCODE_FOR_NAME[name] = opcode
    return op


def _plan_from_mask(mask):
    """Classify each (qtile, kblock-group) region.

    Returns (plan, tiles). plan[j] = tuple of group dicts:
      {'kind': 'full', 'i0', 'w'}                      # no mask
      {'kind': 'ctri', 'i0', 'layout': 'A'|'B'}        # causal diagonal pair
      {'kind': 'data', 'i0', 'w', 'mi', 'y0'}          # arbitrary mask tile
    tiles: list of ('data', np[KB, GSIZE*QT]) mask tiles for 'data' groups.
    """
    plan = []
    tiles = []
    tile_idx = {}
    for j in range(NQT):
        row = []
        mq = mask[j * QT:(j + 1) * QT]  # [QT, S] (q rows, k cols)
        for i0 in range(0, NKB, GSIZE):
            w = min(GSIZE, NKB - i0)
            blk = [mq[:, i * KB:(i + 1) * KB].any() for i in range(i0, i0 + w)]
            if not any(blk):
                continue
            lead = blk.index(True)
            i0 += lead
            w = len(blk) - lead - blk[::-1].index(True)
            R = mq[:, i0 * KB:(i0 + w) * KB]  # [QT, w*KB]
            if R.all():
                row.append({'kind': 'full', 'i0': i0, 'w': w})
                continue
            # causal-diagonal detection: R.T[k - i0*KB, q] == (q_glob >= k_glob)
            qq = np.arange(j * QT, (j + 1) * QT)[None, :]
            kk = np.arange(i0 * KB, (i0 + w) * KB)[:, None]
            rel = i0 * KB - j * QT
            if w == 2 and np.array_equal(R.T, qq >= kk) and rel in (0, 2 * KB):
                row.append({'kind': 'ctri', 'i0': i0,
                            'layout': 'A' if rel == 0 else 'B'})
                continue
            live = np.nonzero(R.any(axis=1))[0]
            y0 = min((int(live[0]) // 256) * 256, QT - 256)
            Mt = R.T.reshape(w, KB, QT).astype(np.float32)
            Mt = np.ascontiguousarray(Mt.transpose(1, 0, 2)).reshape(KB, w * QT)
            key = Mt.tobytes()
            if key not in tile_idx:
                tile_idx[key] = len(tiles)
                tiles.append(('data', Mt))
            row.append({'kind': 'data', 'i0': i0, 'w': w,
                        'mi': tile_idx[key], 'y0': y0})
        # mixed groups first: their longer chains start early
        row.sort(key=lambda e: e['kind'] == 'full')
        plan.append(tuple(row))
    return tuple(plan), tiles


def _stack_mask_tiles(tiles):
    data = [t[1] for t in tiles if t[0] == 'data']
    if not data:
        return None
    out = np.ones((len(data), KB, GSIZE * QT), dtype=np.float32)
    for i, t in enumerate(data):
        out[i, :, :t.shape[1]] = t
    return out


def _group_descs(g):
    """Per-group emission schedule.

    Returns (st_shape, mm1, exp_cols, tris, mm2) where
      mm1 = [(gp, st_off, n, q0)]     st[:, st_off:st_off+n] = k[gp] x q[:, q0:]
      tris = [(pt_off,)]              128-wide triangle multiply offsets
      mm2 = [(gp, rhs_off, n, q0)]    acc[:, q0:] += v[gp] x pt[:, rhs_off:+n]
    """
    if g['kind'] == 'full':
        w = g['w']
        mm1 = [(gp, gp * QT, QT, 0) for gp in range(w)]
        mm2 = [(gp, gp * QT, QT, 0) for gp in range(w)]
        return (w * QT, mm1, w * QT, [], mm2)
    if g['kind'] == 'ctri':
        if g['layout'] == 'A':
            # blocks at rel 0, 128: block0 live q[0:512), block1 live q[128:512)
            mm1 = [(0, 0, QT, 0), (1, QT + KB, QT - KB, KB)]
            tris = [0, QT + KB]
            mm2 = [(0, 0, QT, 0), (1, QT + KB, QT - KB, KB)]
            return (2 * QT, mm1, 2 * QT, tris, mm2)
        # layout B: blocks at rel 256, 384; compact [128, 512] tile:
        # block0 (live q[256:512)) at cols [0:256); block1 (live q[384:512))
        # at cols [256:512) of which [256:384) is dead (never read by mm2).
        mm1 = [(0, 0, 256, 256), (1, 256, 256, 256)]
        tris = [0, 384]
        mm2 = [(0, 0, 256, 256), (1, 384, 128, 384)]
        return (QT, mm1, QT, tris, mm2)
    # data fallback: baseline scheme (y0 trim, full-region mask mult)
    w, y0 = g['w'], g['y0']
    mm1 = [(gp, gp * QT + y0, QT - y0, y0) for gp in range(w)]
    mm2 = [(gp, gp * QT + y0, QT - y0, y0) for gp in range(w)]
    return (w * QT, mm1, w * QT - w * y0, [], mm2)


def _assign_engines(plan):
    """LPT-balance exp + copy instructions across ScalarE / VectorE.

    Returns (exp_eng[(pair,j,gi,sub)], copy_eng[(pair,j,sub)]).
    """
    items = []  # (key, cost_scalar_ns, cost_dve_ns)
    dve_pre = 0.0
    for j, groups in enumerate(plan):
        for gi, g in enumerate(groups):
            _, _, exp_cols, tris, _ = _group_descs(g)
            for pair in range(HPC // 2):
                for sub in range(2):
                    items.append((('e', pair, j, gi, sub),
                                  exp_cols / 1.2 + 185.0,
                                  exp_cols / 0.96 + 125.0))
            dve_pre += len(tris) * (KB / 0.96 + 60.0) * 2 * (HPC // 2)
            if g['kind'] == 'data':
                dve_pre += (g['w'] * QT) / 0.96 * 2 * (HPC // 2)
        for pair in range(HPC // 2):
            for sub in range(2):
                items.append((('c', pair, j, sub),
                              QT / 1.2 + 185.0,
                              QT / 0.96 + 125.0))
    load = {SC: 1283.0, DV: dve_pre}
    assign = {}
    # exp instrs: greedy in emission order (adjacent groups' exps then
    # alternate between ScalarE/VectorE and overlap, halving the mm2 feed
    # latency) while tracking global balance.
    exps = [it for it in items if it[0][0] == 'e']
    exps.sort(key=lambda it: (it[0][2], it[0][3], it[0][1], it[0][4]))
    for key, cs, cd in exps:
        if load[SC] + cs <= load[DV] + cd:
            assign[key] = SC
            load[SC] += cs
        else:
            assign[key] = DV
            load[DV] += cd
    # copies: LPT onto the lighter engine
    for key, cs, cd in items:
        if key[0] == 'c':
            if load[SC] + cs <= load[DV] + cd:
                assign[key] = SC
                load[SC] += cs
            else:
                assign[key] = DV
                load[DV] += cd
    return assign, load


def _build(plan, tiles, repeats=1, bufs=None):
    from contextlib import ExitStack

    import concourse.tile as tile
    from concourse import bacc, mybir

    f32 = mybir.dt.float32
    f32r = mybir.dt.float32r
    exp_op = _register_exp_op()

    bufs = bufs or {'st': 3, 'acc': 1, 'pt': 6}

    nc = bacc.Bacc("TRN2", target_bir_lowering=False, debug=False,
                   num_devices=NCORES)

    qt_d = nc.dram_tensor("qt", [HPC // 2, 128, S], f32r, kind="ExternalInput").ap()
    kt_d = nc.dram_tensor("kt", [HPC // 2, 128, S], f32r, kind="ExternalInput").ap()
    v_d = nc.dram_tensor("v", [HPC, 128, NKB * (D + 1)], f32r,
                         kind="ExternalInput").ap()
    out_d = nc.dram_tensor("out", [HPC, D + 1, S], f32, kind="ExternalOutput").ap()
    data_idx = {}
    for ti, t in enumerate(tiles):
        if t[0] == 'data':
            data_idx[ti] = len(data_idx)
    n_mtiles = len(data_idx)
    if n_mtiles:
        mt_d = nc.dram_tensor("mt", [n_mtiles, KB, GSIZE * QT], f32,
                              kind="ExternalInput").ap()
    resident = n_mtiles <= MAX_RESIDENT_MASKS

    assign, _ = _assign_engines(plan)
    has_tri = any(g['kind'] == 'ctri' for row in plan for g in row)

    with tile.TileContext(nc) as tc, ExitStack() as ctx:
        qk_pool = ctx.enter_context(tc.tile_pool(name="qk", bufs=2))
        v_pool = ctx.enter_context(tc.tile_pool(name="vp", bufs=4))
        st_pool = ctx.enter_context(
            tc.tile_pool(name="st", bufs=bufs['st'], space="PSUM"))
        pt_pool = ctx.enter_context(tc.tile_pool(name="pt", bufs=bufs['pt']))
        acc_pool = ctx.enter_context(
            tc.tile_pool(name="acc", bufs=bufs['acc'], space="PSUM"))
        out_pool = ctx.enter_context(tc.tile_pool(name="ob", bufs=3))
        m_pool = ctx.enter_context(
            tc.tile_pool(name="mt", bufs=1 if resident else 2))

        bias_t = m_pool.tile([128, 1], f32, tag="bias", name="bias")
        nc.gpsimd.memset(bias_t[:], float(8.0 * PLOGC))

        tri_t = None
        if has_tri:
            # TRI[p, y] = 1 where y >= p else 0  ([128, 128])
            tri_t = m_pool.tile([KB, KB], f32, tag="tri", name="tri")
            nc.gpsimd.memset(tri_t[:], 1.0)
            nc.gpsimd.affine_select(
                out=tri_t[:], in_=tri_t[:],
                compare_op=mybir.AluOpType.is_ge,
                fill=0.0, base=0,
                pattern=[[1, KB]],
                channel_multiplier=-1)

        m_tiles = {}

        def _preload_masks():
            if n_mtiles and resident:
                for di in range(n_mtiles):
                    m = m_pool.tile([KB, GSIZE * QT], f32, tag=f"m{di}",
                                    name=f"md{di}")
                    nc.sync.dma_start(m[:], mt_d[di])
                    m_tiles[di] = m

        for rep in range(repeats):
          for pair in range(HPC // 2):
            kt_c, qt_c = [], []
            v_ts = []
            out_sbs = []
            for c in range(NQT):
                kt1 = qk_pool.tile([128, QT], f32r, tag=f"kt{c}", name=f"kt{pair}_{c}")
                nc.sync.dma_start(kt1[:], kt_d[pair, :, c * QT:(c + 1) * QT])
                kt_c.append(kt1)
                qt1 = qk_pool.tile([128, QT], f32r, tag=f"qt{c}", name=f"qt{pair}_{c}")
                nc.sync.dma_start(qt1[:], qt_d[pair, :, c * QT:(c + 1) * QT])
                qt_c.append(qt1)
                if c == 1:
                    # v is first needed by mm2 (after mm1+exp of group 0);
                    # loading it after the chunk-1 k/q DMAs lets the first
                    # mm1 start ~1.5us earlier
                    for sub in range(2):
                        h = 2 * pair + sub
                        v_t = v_pool.tile([128, NKB * (D + 1)], f32r, tag="v",
                                          name=f"v{h}")
                        nc.sync.dma_start(v_t[:], v_d[h])
                        v_ts.append(v_t)
                        out_sbs.append(
                            out_pool.tile([D + 1, S], f32, tag="o", name=f"ob{h}"))
                    if pair == 0:
                        _preload_masks()

            for j in range(NQT):
                groups = plan[j]
                accs = [acc_pool.tile([D + 1, QT], f32, tag=f"a{sub}",
                                      name=f"acc{sub}")
                        for sub in range(2)]
                started = [False, False]

                def _emit_mm2(gi, g, pts, last):
                    _, _, _, _, mm2 = _group_descs(g)
                    for sub in range(2):
                        pt = pts[sub]
                        v_t = v_ts[sub]
                        acc = accs[sub]
                        for mi, (gp, rhs_off, n, q0) in enumerate(mm2):
                            i = g['i0'] + gp
                            nc.tensor.matmul(
                                acc[:, q0:q0 + n],
                                lhsT=v_t[:, i * (D + 1):(i + 1) * (D + 1)],
                                rhs=pt[:, rhs_off:rhs_off + n],
                                start=not started[sub],
                                stop=last and mi == len(mm2) - 1,
                                skip_group_check=True)
                            started[sub] = True

                pending = None  # (gi, g, pts) with mm2 not yet emitted
                for gi, g in enumerate(groups):
                    st_cols, mm1, exp_cols, tris, mm2 = _group_descs(g)
                    sts, pts = [], []
                    # both heads' mm1s adjacent -> concurrent PE row-tiles
                    for sub in range(2):
                        po = 64 * sub
                        st = st_pool.tile([128, st_cols], f32, tag="s",
                                          name=f"st{sub}")
                        sts.append(st)
                        for gp, st_off, n, q0 in mm1:
                            i = g['i0'] + gp
                            nc.tensor.matmul(
                                st[:, st_off:st_off + n],
                                lhsT=kt_c[i // 4][po:po + 64,
                                                  (i % 4) * KB:(i % 4 + 1) * KB],
                                rhs=qt_c[j][po:po + 64, q0:q0 + n],
                                start=True, stop=True)
                    for sub in range(2):
                        st = sts[sub]
                        pt = pt_pool.tile([128, st_cols], f32r, tag="p",
                                          name=f"pt{sub}")
                        pts.append(pt)
                        eng = assign[('e', pair, j, gi, sub)]
                        if g['kind'] == 'data' and g['y0'] > 0:
                            w, y0 = g['w'], g['y0']
                            st_ap = st[:].rearrange(
                                "p (g y) -> p g y", y=QT)[:, :w, y0:]
                            pt_ap = pt[:].rearrange(
                                "p (g y) -> p g y", y=QT)[:, :w, y0:]
                        else:
                            st_ap = st[:, :]
                            pt_ap = pt[:, :]
                        if eng == SC:
                            nc.scalar.activation(
                                pt_ap, st_ap,
                                mybir.ActivationFunctionType.Exp,
                                bias=bias_t[:], scale=8.0)
                        else:
                            nc.vector._custom_dve(
                                exp_op, out=pt_ap, in0=st_ap,
                                s0=float(PA0), s1=float(PA1), imm2=float(PA2))
                        for off in tris:
                            nc.vector.tensor_mul(
                                pt[:, off:off + KB], pt[:, off:off + KB],
                                tri_t[:, :])
                        if g['kind'] == 'data':
                            m_t = m_tiles.get(g['mi'])
                            if m_t is None:
                                m_t = m_pool.tile([KB, GSIZE * QT], f32, tag="ms")
                                nc.sync.dma_start(m_t[:], mt_d[data_idx[g['mi']]])
                            w, y0 = g['w'], g['y0']
                            if y0 == 0:
                                m_ap = m_t[:, :w * QT]
                            else:
                                m_ap = m_t[:].rearrange(
                                    "p (g y) -> p g y", y=QT)[:, :w, y0:]
                            nc.vector.tensor_mul(pt_ap, pt_ap, m_ap)
                    if pending is not None:
                        _emit_mm2(*pending, last=False)
                    pending = (gi, g, pts)
                if pending is not None:
                    _emit_mm2(*pending, last=True)
                for sub in range(2):
                    osl = out_sbs[sub][:, j * QT:(j + 1) * QT]
                    if groups:
                        eng = assign[('c', pair, j, sub)]
                        if eng == SC:
                            nc.scalar.copy(osl, accs[sub][:])
                        else:
                            nc.vector.tensor_copy(osl, accs[sub][:])
                    else:
                        nc.vector.memset(osl, 0.0)
                    nc.sync.dma_start(
                        out_d[2 * pair + sub, :, j * QT:(j + 1) * QT], osl)

    nc.compile()
    return nc


def _get_nc(mask):
    key = mask.tobytes()
    if key not in _CACHE:
        plan, mtiles = _plan_from_mask(mask)
        nc = _build(plan, mtiles)
        _CACHE[key] = (nc, mtiles)
    return _CACHE[key]


def kernel(q, k, v, mask, _trace=False):
    from concourse.bass_utils import run_bass_kernel_spmd

    mask = np.asarray(mask).astype(bool)
    q = np.asarray(q, dtype=np.float32).reshape(B * H, S, D)
    k = np.asarray(k, dtype=np.float32).reshape(B * H, S, D)
    v = np.asarray(v, dtype=np.float32).reshape(B * H, S, D)
    q = q * np.float32(1.0 / (8.0 * np.sqrt(D)))  # PSUM holds r = score/8

    nc, mtiles = _get_nc(mask)
    mt = _stack_mask_tiles(mtiles)

    in_maps = []
    for c in range(NCORES):
        sl = slice(HPC * c, HPC * (c + 1))
        qc = np.ascontiguousarray(q[sl].transpose(0, 2, 1)).reshape(HPC // 2, 128, S)
        kc = np.ascontiguousarray(k[sl].transpose(0, 2, 1)).reshape(HPC // 2, 128, S)
        vc = np.concatenate(
            [v[sl], np.ones((HPC, S, 1), dtype=np.float32)], axis=2)
        vc = vc.reshape(HPC, NKB, KB, D + 1).transpose(0, 2, 1, 3)
        vc = np.ascontiguousarray(vc).reshape(HPC, KB, NKB * (D + 1))
        m = {"qt": qc, "kt": kc, "v": vc}
        if mt is not None:
            m["mt"] = mt
        in_maps.append(m)

    res = run_bass_kernel_spmd(nc, in_maps, core_ids=list(range(NCORES)),
                               trace=_trace)

    outs = []
    for c in range(NCORES):
        o = res.results[c]["out"]  # [HPC, D+1, S]
        num = o[:, :D, :]
        den = o[:, D:D + 1, :]
        with np.errstate(invalid='ignore', divide='ignore'):
            outs.append((num / den).transpose(0, 2, 1))  # [HPC, S, D]
    full = np.concatenate(outs, axis=0).reshape(B, H, S, D).astype(np.float32)
    if _trace:
        return full, res
    return full
